# revision 8
# baseline (speedup 1.0000x reference)
"""Trainium2 Bass kernel for MaxTimesPlusErosionLiftingP4 — v2 (g-sharing).

Key idea vs v1: the four group rotations share one set of 147 affine
tap-images.  For rotation i, output pixel z:
    out_i[z] = sum_c min_q g_q[z + s_i(q)]
where g_q = (xpad - k_ero[q]) * inv_t_ero[q] (per c, f) and the shift
s_i(q) runs over the 7x7 kernel support rotated by i.  So the affine
images g_q are produced ONCE (147 ScalarE activation ops over a 14x70
per-block slab instead of 147 ops over the full 32x64 pixel range = 4x
less ScalarE work), and the per-rotation structure moves into the DVE
min-accumulate reads (strided views of g).

Device layout: 128 partitions = 4 row-blocks x 32 filters.  Each core
takes 32 of the 256 (b,h) output rows; each partition-block covers 8 of
them.  Per-partition slab = [C=3, 14, 70] fp16 (8 rows + 3 halo each
side, 64 cols + 3 pad each side).  Rotations 0/2 and 1/3 are 180-degree
partners, so their two min-accumulates fuse into one 5-dim
tensor_tensor (pair dim stride = offset difference).  Channel sum on
DVE, DMA'd out; host reassembles [B,4,H,W,F].
"""

import os
from contextlib import ExitStack

import numpy as np

import concourse.bacc as bacc
import concourse.bass as bass
import concourse.mybir as mybir
import concourse.tile as tile
from concourse.bass_utils import run_bass_kernel_spmd

B, H, W, C, F = 4, 64, 64, 3, 32
KH = KW = 7
P = KH * KW  # 49 taps
NCORES = 8
ROWS = (B * H) // NCORES  # 32 output rows per core
NBLK = 4
BROWS = ROWS // NBLK  # 8 rows per partition-block
HSLAB = BROWS + KH - 1  # 14 slab rows
WSLAB = W + KW - 1  # 70 slab cols
SLAB = C * HSLAB * WSLAB  # 2940 per partition
NUNITS = P * C  # 147
EPS = 1e-7

_DT = os.environ.get("EROSION_DT", "fp16")
_REPEAT = int(os.environ.get("EROSION_REPEAT", 1))
_GBUFS = int(os.environ.get("EROSION_GBUFS", 10))
_PAIR = int(os.environ.get("EROSION_PAIR", 0))  # fuse 180-deg rotation pairs
# (measured: the hand-built 5-dim pair view crashes the exec unit — keep 0)
_SUM16 = int(os.environ.get("EROSION_SUM16", 1))  # fp16 channel sum + output
_PDVE = int(os.environ.get("EROSION_PDVE", 6))  # producer units on DVE
_PGPS = int(os.environ.get("EROSION_PGPS", 0))  # producer units on GpSimd
_NOMIN = int(os.environ.get("EROSION_NOMIN", 0))  # diag: producers only
_BENCHOUT = int(os.environ.get("EROSION_BENCHOUT", 0))  # tiny output (bench only)

_cache = {}

last_results = None


def _shifts():
    """SH[i][t] = (i', j') top-left of the 8x64 view for rotation i, tap t
    (t indexes k_ero row-major: t = a*7 + b)."""
    idx = np.arange(P).reshape(KH, KW)
    sh = [[None] * P for _ in range(4)]
    for i in range(4):
        m = np.rot90(idx, i)
        for ip in range(KH):
            for jp in range(KW):
                sh[i][int(m[ip, jp])] = (ip, jp)
    return sh


SH = _shifts()


def _spread(total, count):
    return [((i + 1) * count) // total > (i * count) // total for i in range(total)]


def _build_module():
    dt = mybir.dt.float16 if _DT == "fp16" else mybir.dt.float32
    f32 = mybir.dt.float32
    sum_dt = dt if _SUM16 else f32

    nc = bacc.Bacc("TRN2", target_bir_lowering=False, debug=False)
    xs_d = nc.dram_tensor("xs", [NBLK * SLAB], dt, kind="ExternalInput")
    tabs_d = nc.dram_tensor("tabs", [128, 2 * NUNITS], f32, kind="ExternalInput")
    out_shape = [128, 64] if _BENCHOUT else [128, 4 * BROWS * W]
    out_d = nc.dram_tensor("out", out_shape, sum_dt, kind="ExternalOutput")

    prod_dve = _spread(NUNITS, _PDVE)
    rest = [j for j in range(NUNITS) if not prod_dve[j]]
    gps_in_rest = _spread(len(rest), _PGPS)
    prod_gps = [False] * NUNITS
    for pos, j in enumerate(rest):
        if gps_in_rest[pos]:
            prod_gps[j] = True

    with tile.TileContext(nc) as tc, ExitStack() as ctx:
        singles = ctx.enter_context(tc.tile_pool(name="singles", bufs=1))
        gpool = ctx.enter_context(tc.tile_pool(name="g", bufs=_GBUFS))
        spool = ctx.enter_context(tc.tile_pool(name="s", bufs=2))

        slab = singles.tile([128, SLAB], dt, tag="slab", name="slab")
        tabs = singles.tile([128, 2 * NUNITS], f32, tag="tabs", name="tabs")
        # acc[:, i] = running per-channel min for rotation i
        acc = singles.tile([128, 4, C, BROWS, W], dt, tag="acc", name="acc")
        osum = singles.tile([128, 4, BROWS, W], sum_dt, tag="osum", name="osum")

        # input DMAs: per-block slab broadcast to its 32 filter partitions
        for blk in range(NBLK):
            eng = nc.sync if blk % 2 == 0 else nc.scalar
            eng.dma_start(
                out=slab[blk * F : (blk + 1) * F],
                in_=bass.AP(tensor=xs_d, offset=blk * SLAB, ap=[[0, F], [1, SLAB]]),
            )
        nc.sync.dma_start(out=tabs[:], in_=tabs_d.ap())
        rtab = tabs[:, 0:NUNITS]
        btab = tabs[:, NUNITS : 2 * NUNITS]

        slab_r = slab[:].rearrange("p (c h w) -> p c h w", c=C, h=HSLAB, w=WSLAB)

        if _NOMIN:
            nc.vector.memset(osum[:], 0.0)

        def pair_view(g_r, t, i):
            """5D view covering rotations i and i+2 of tap t."""
            ip0, jp0 = SH[i][t]
            ip2, jp2 = SH[i + 2][t]
            o0 = 70 * ip0 + jp0
            delta = (70 * ip2 + jp2) - o0
            v = g_r[:, :, ip0 : ip0 + BROWS, jp0 : jp0 + W]
            w = v.copy()
            w.ap = mybir.VecI64Pair(
                [list(v.ap[0]), [delta, 2]] + [list(d) for d in v.ap[1:]]
            )
            return w

        for _rep in range(_REPEAT):
            first_views = [None] * 4
            for t in range(P):
                g = gpool.tile([128, SLAB], dt, tag="g", name="g")
                g_r = g[:].rearrange("p (c h w) -> p c h w", c=C, h=HSLAB, w=WSLAB)
                for c in range(C):
                    j = t * C + c
                    sr = rtab[:, j : j + 1]
                    sb = btab[:, j : j + 1]
                    if prod_dve[j]:
                        nc.vector.tensor_scalar(
                            g_r[:, c], slab_r[:, c], sr, sb,
                            mybir.AluOpType.mult, mybir.AluOpType.add,
                        )
                    elif prod_gps[j]:
                        nc.gpsimd.tensor_scalar(
                            g_r[:, c], slab_r[:, c], sr, sb,
                            mybir.AluOpType.mult, mybir.AluOpType.add,
                        )
                    else:
                        nc.scalar.activation(
                            out=g_r[:, c], in_=slab_r[:, c],
                            func=mybir.ActivationFunctionType.Identity,
                            bias=sb, scale=sr,
                        )
                if _NOMIN:
                    continue
                if _PAIR:
                    # _PAIR=2: merge a rotation pair only when the pair-dim
                    # stride is positive (negative strides suspected in the
                    # exec-unit crash); singles otherwise.  t<2 always single.
                    for i in range(4):
                        ip, jp = SH[i][t]
                        src = g_r[:, :, ip : ip + BROWS, jp : jp + W]
                        if t == 0:
                            first_views[i] = src
                        elif t == 1:
                            nc.vector.tensor_tensor(
                                acc[:, i], first_views[i], src, mybir.AluOpType.min
                            )
                    if t >= 2:
                        for i in range(2):
                            ip0, jp0 = SH[i][t]
                            ip2, jp2 = SH[i + 2][t]
                            delta = (70 * ip2 + jp2) - (70 * ip0 + jp0)
                            if _PAIR == 1 or delta > 0:
                                src = pair_view(g_r, t, i)
                                dst = acc[:, i : i + 3 : 2]
                                nc.vector.tensor_tensor(
                                    dst, dst, src, mybir.AluOpType.min
                                )
                            else:
                                for ii in (i, i + 2):
                                    ip, jp = SH[ii][t]
                                    src = g_r[
                                        :, :, ip : ip + BROWS, jp : jp + W
                                    ]
                                    nc.vector.tensor_tensor(
                                        acc[:, ii], acc[:, ii], src,
                                        mybir.AluOpType.min,
                                    )
                else:
                    for i in range(4):
                        ip, jp = SH[i][t]
                        src = g_r[:, :, ip : ip + BROWS, jp : jp + W]
                        if t == 0:
                            first_views[i] = src
                        elif t == 1:
                            nc.vector.tensor_tensor(
                                acc[:, i], first_views[i], src, mybir.AluOpType.min
                            )
                        else:
                            nc.vector.tensor_tensor(
                                acc[:, i], acc[:, i], src, mybir.AluOpType.min
                            )

            if _NOMIN:
                continue
            # channel sum: osum = acc[c0] + acc[c1] + acc[c2]
            s01 = spool.tile([128, 4, BROWS, W], sum_dt, tag="s01", name="s01")
            nc.vector.tensor_tensor(
                s01[:], acc[:, :, 0], acc[:, :, 1], mybir.AluOpType.add
            )
            nc.vector.tensor_tensor(
                osum[:], s01[:], acc[:, :, 2], mybir.AluOpType.add
            )

        osum_flat = osum[:].rearrange("p a b c -> p (a b c)")
        if _BENCHOUT:
            nc.sync.dma_start(out=out_d.ap(), in_=osum_flat[:, :64])
        else:
            nc.sync.dma_start(out=out_d.ap(), in_=osum_flat)

    nc.compile()
    return nc


def _get_module():
    key = (_DT, _REPEAT, _GBUFS, _PAIR, _SUM16, _PDVE, _NOMIN, _BENCHOUT)
    if key not in _cache:
        _cache[key] = _build_module()
    return _cache[key]


def _host_tables(kernel, timesKernel):
    """tabs[p, j] = r; tabs[p, 147+j] = -k*r for unit j = t*C + c,
    t in k_ero row-major coords; p = blk*32 + f (f-dependent only)."""
    k_ero = np.rot90(kernel, 2, axes=(0, 1)).reshape(P, C, F)
    t_ero = np.rot90(timesKernel, 2, axes=(0, 1)).reshape(P, C, F)
    R = (1.0 / (t_ero + np.float32(EPS))).astype(np.float32)  # [P,C,F]
    Bt = (-k_ero * R).astype(np.float32)
    tabs = np.zeros((128, 2 * NUNITS), np.float32)
    for blk in range(NBLK):
        sl = slice(blk * F, (blk + 1) * F)
        tabs[sl, :NUNITS] = R.reshape(NUNITS, F).T
        tabs[sl, NUNITS:] = Bt.reshape(NUNITS, F).T
    return tabs


def _host_slabs(x):
    """[NCORES, NBLK*SLAB] fp16: per core, 4 block slabs [C, 14, 70]."""
    np_dt = np.float16 if _DT == "fp16" else np.float32
    out = np.zeros((NCORES, NBLK, C, HSLAB, WSLAB), np.float32)
    pad = (KH - 1) // 2
    for m in range(NCORES):
        b, half = divmod(m, 2)
        h0 = half * ROWS
        for blk in range(NBLK):
            r0 = h0 + blk * BROWS - pad
            lo, hi = max(r0, 0), min(r0 + HSLAB, H)
            out[m, blk, :, lo - r0 : hi - r0, pad : pad + W] = np.transpose(
                x[b, lo:hi, :, :], (2, 0, 1)
            )
    return out.reshape(NCORES, NBLK * SLAB).astype(np_dt)


def emulate(x, kernel, timesKernel):
    """Pure-numpy emulation of the device math (fp32; layout-faithful)."""
    tabs = _host_tables(kernel, timesKernel)
    slabs = _host_slabs(np.asarray(x, np.float32)).astype(np.float32)
    full = np.zeros((B, 4, H, W, F), np.float32)
    for m in range(NCORES):
        b, half = divmod(m, 2)
        h0 = half * ROWS
        sl = slabs[m].reshape(NBLK, C, HSLAB, WSLAB)
        acc = np.full((4, NBLK, C, F, BROWS, W), np.inf, np.float32)
        for t in range(P):
            for c in range(C):
                j = t * C + c
                r = tabs[:F, j]
                bt = tabs[:F, NUNITS + j]
                g = (
                    sl[:, c, None, :, :] * r[None, :, None, None]
                    + bt[None, :, None, None]
                )
                for i in range(4):
                    ip, jp = SH[i][t]
                    acc[i, :, c] = np.minimum(
                        acc[i, :, c], g[:, :, ip : ip + BROWS, jp : jp + W]
                    )
        o = acc.sum(axis=2)
        for blk in range(NBLK):
            full[b, :, h0 + blk * BROWS : h0 + (blk + 1) * BROWS, :, :] = (
                np.transpose(o[:, blk], (0, 2, 3, 1))
            )
    return full


def kernel(x, kernel, timesKernel):
    global last_results
    x = np.asarray(x, np.float32)
    kernel = np.asarray(kernel, np.float32)
    timesKernel = np.asarray(timesKernel, np.float32)

    tabs = _host_tables(kernel, timesKernel)
    slabs = _host_slabs(x)

    nc = _get_module()
    in_maps = [{"xs": slabs[m], "tabs": tabs} for m in range(NCORES)]
    res = run_bass_kernel_spmd(nc, in_maps, list(range(NCORES)))
    last_results = res

    full = np.zeros((B, 4, H, W, F), np.float32)
    for m in range(NCORES):
        b, half = divmod(m, 2)
        h0 = half * ROWS
        o = res.results[m]["out"].astype(np.float32).reshape(NBLK, F, 4, BROWS, W)
        for blk in range(NBLK):
            full[b, :, h0 + blk * BROWS : h0 + (blk + 1) * BROWS, :, :] = (
                np.transpose(o[blk], (1, 2, 3, 0))
            )
    return full


# revision 13
# speedup vs baseline: 4.5524x; 4.5524x over previous
"""Trainium2 Bass kernel for MaxTimesPlusErosionLiftingP4 — v2 (g-sharing).

Key idea vs v1: the four group rotations share one set of 147 affine
tap-images.  For rotation i, output pixel z:
    out_i[z] = sum_c min_q g_q[z + s_i(q)]
where g_q = (xpad - k_ero[q]) * inv_t_ero[q] (per c, f) and the shift
s_i(q) runs over the 7x7 kernel support rotated by i.  So the affine
images g_q are produced ONCE (147 ScalarE activation ops over a 14x70
per-block slab instead of 147 ops over the full 32x64 pixel range = 4x
less ScalarE work), and the per-rotation structure moves into the DVE
min-accumulate reads (strided views of g).

Device layout: 128 partitions = 4 row-blocks x 32 filters.  Each core
takes 32 of the 256 (b,h) output rows; each partition-block covers 8 of
them.  Per-partition slab = [C=3, 14, 70] fp16 (8 rows + 3 halo each
side, 64 cols + 3 pad each side).  Per tap: 3 producer ops (ScalarE
activation Identity with per-partition scale/bias; ~12 of the 147 units
spread to GpSimd tensor_scalar for balance) then 4 DVE min-accumulates
(one per rotation, [128, 3, 8, 64] strided views of g).  Channel sum in
fp16 on DVE, DMA'd out; host reassembles [B,4,H,W,F].
"""

import os
from contextlib import ExitStack

import numpy as np

import concourse.bacc as bacc
import concourse.bass as bass
import concourse.mybir as mybir
import concourse.tile as tile
from concourse.bass_utils import run_bass_kernel_spmd

B, H, W, C, F = 4, 64, 64, 3, 32
KH = KW = 7
P = KH * KW  # 49 taps
NCORES = 8
ROWS = (B * H) // NCORES  # 32 output rows per core
NBLK = 4
BROWS = ROWS // NBLK  # 8 rows per partition-block
HSLAB = BROWS + KH - 1  # 14 slab rows
WSLAB = W + KW - 1  # 70 slab cols
SLAB = C * HSLAB * WSLAB  # 2940 per partition
NUNITS = P * C  # 147
EPS = 1e-7

_DT = os.environ.get("EROSION_DT", "fp16")
_REPEAT = int(os.environ.get("EROSION_REPEAT", 1))
_GBUFS = int(os.environ.get("EROSION_GBUFS", 10))
# (removed: fusing 180-deg rotation pairs into one 5-dim tensor_tensor is
# impossible — the DVE TT ISA static pattern is TENSOR3D, max 3 free dims;
# walrus mis-folds deeper APs and the kernel crashes the exec unit)
_SUM16 = int(os.environ.get("EROSION_SUM16", 1))  # fp16 channel sum + output
_PDVE = int(os.environ.get("EROSION_PDVE", 0))  # producer units on DVE
_PGPS = int(os.environ.get("EROSION_PGPS", 12))  # producer units on GpSimd
_NOMIN = int(os.environ.get("EROSION_NOMIN", 0))  # diag: producers only
_BENCHOUT = int(os.environ.get("EROSION_BENCHOUT", 0))  # tiny output (bench only)

_cache = {}

last_results = None


def _shifts():
    """SH[i][t] = (i', j') top-left of the 8x64 view for rotation i, tap t
    (t indexes k_ero row-major: t = a*7 + b)."""
    idx = np.arange(P).reshape(KH, KW)
    sh = [[None] * P for _ in range(4)]
    for i in range(4):
        m = np.rot90(idx, i)
        for ip in range(KH):
            for jp in range(KW):
                sh[i][int(m[ip, jp])] = (ip, jp)
    return sh


SH = _shifts()


def _spread(total, count):
    return [((i + 1) * count) // total > (i * count) // total for i in range(total)]


def _build_module():
    dt = mybir.dt.float16 if _DT == "fp16" else mybir.dt.float32
    f32 = mybir.dt.float32
    sum_dt = dt if _SUM16 else f32

    nc = bacc.Bacc("TRN2", target_bir_lowering=False, debug=False)
    xs_d = nc.dram_tensor("xs", [NBLK * SLAB], dt, kind="ExternalInput")
    tabs_d = nc.dram_tensor("tabs", [128, 2 * NUNITS], f32, kind="ExternalInput")
    out_shape = [128, 64] if _BENCHOUT else [128, 4 * BROWS * W]
    out_d = nc.dram_tensor("out", out_shape, sum_dt, kind="ExternalOutput")

    prod_dve = _spread(NUNITS, _PDVE)
    rest = [j for j in range(NUNITS) if not prod_dve[j]]
    gps_in_rest = _spread(len(rest), _PGPS)
    prod_gps = [False] * NUNITS
    for pos, j in enumerate(rest):
        if gps_in_rest[pos]:
            prod_gps[j] = True

    with tile.TileContext(nc) as tc, ExitStack() as ctx:
        singles = ctx.enter_context(tc.tile_pool(name="singles", bufs=1))
        gpool = ctx.enter_context(tc.tile_pool(name="g", bufs=_GBUFS))
        spool = ctx.enter_context(tc.tile_pool(name="s", bufs=2))

        slab = singles.tile([128, SLAB], dt, tag="slab", name="slab")
        tabs = singles.tile([128, 2 * NUNITS], f32, tag="tabs", name="tabs")
        # acc[:, i] = running per-channel min for rotation i
        acc = singles.tile([128, 4, C, BROWS, W], dt, tag="acc", name="acc")
        osum = singles.tile([128, 4, BROWS, W], sum_dt, tag="osum", name="osum")

        # input DMAs: per-block slab broadcast to its 32 filter partitions
        for blk in range(NBLK):
            eng = nc.sync if blk % 2 == 0 else nc.scalar
            eng.dma_start(
                out=slab[blk * F : (blk + 1) * F],
                in_=bass.AP(tensor=xs_d, offset=blk * SLAB, ap=[[0, F], [1, SLAB]]),
            )
        nc.sync.dma_start(out=tabs[:], in_=tabs_d.ap())
        rtab = tabs[:, 0:NUNITS]
        btab = tabs[:, NUNITS : 2 * NUNITS]

        slab_r = slab[:].rearrange("p (c h w) -> p c h w", c=C, h=HSLAB, w=WSLAB)

        if _NOMIN:
            nc.vector.memset(osum[:], 0.0)

        for _rep in range(_REPEAT):
            first_views = [None] * 4
            for t in range(P):
                g = gpool.tile([128, SLAB], dt, tag="g", name="g")
                g_r = g[:].rearrange("p (c h w) -> p c h w", c=C, h=HSLAB, w=WSLAB)
                for c in range(C):
                    j = t * C + c
                    sr = rtab[:, j : j + 1]
                    sb = btab[:, j : j + 1]
                    if prod_dve[j]:
                        nc.vector.tensor_scalar(
                            g_r[:, c], slab_r[:, c], sr, sb,
                            mybir.AluOpType.mult, mybir.AluOpType.add,
                        )
                    elif prod_gps[j]:
                        nc.gpsimd.tensor_scalar(
                            g_r[:, c], slab_r[:, c], sr, sb,
                            mybir.AluOpType.mult, mybir.AluOpType.add,
                        )
                    else:
                        nc.scalar.activation(
                            out=g_r[:, c], in_=slab_r[:, c],
                            func=mybir.ActivationFunctionType.Identity,
                            bias=sb, scale=sr,
                        )
                if _NOMIN:
                    continue
                for i in range(4):
                    ip, jp = SH[i][t]
                    src = g_r[:, :, ip : ip + BROWS, jp : jp + W]
                    if t == 0:
                        first_views[i] = src
                    elif t == 1:
                        nc.vector.tensor_tensor(
                            acc[:, i], first_views[i], src, mybir.AluOpType.min
                        )
                    else:
                        nc.vector.tensor_tensor(
                            acc[:, i], acc[:, i], src, mybir.AluOpType.min
                        )

            if _NOMIN:
                continue
            # channel sum: osum = acc[c0] + acc[c1] + acc[c2]
            s01 = spool.tile([128, 4, BROWS, W], sum_dt, tag="s01", name="s01")
            nc.vector.tensor_tensor(
                s01[:], acc[:, :, 0], acc[:, :, 1], mybir.AluOpType.add
            )
            nc.vector.tensor_tensor(
                osum[:], s01[:], acc[:, :, 2], mybir.AluOpType.add
            )

        osum_flat = osum[:].rearrange("p a b c -> p (a b c)")
        if _BENCHOUT:
            nc.sync.dma_start(out=out_d.ap(), in_=osum_flat[:, :64])
        else:
            nc.sync.dma_start(out=out_d.ap(), in_=osum_flat)

    nc.compile()
    return nc


def _get_module():
    key = (_DT, _REPEAT, _GBUFS, _SUM16, _PDVE, _PGPS, _NOMIN, _BENCHOUT)
    if key not in _cache:
        _cache[key] = _build_module()
    return _cache[key]


def _host_tables(kernel, timesKernel):
    """tabs[p, j] = r; tabs[p, 147+j] = -k*r for unit j = t*C + c,
    t in k_ero row-major coords; p = blk*32 + f (f-dependent only)."""
    k_ero = np.rot90(kernel, 2, axes=(0, 1)).reshape(P, C, F)
    t_ero = np.rot90(timesKernel, 2, axes=(0, 1)).reshape(P, C, F)
    R = (1.0 / (t_ero + np.float32(EPS))).astype(np.float32)  # [P,C,F]
    Bt = (-k_ero * R).astype(np.float32)
    tabs = np.zeros((128, 2 * NUNITS), np.float32)
    for blk in range(NBLK):
        sl = slice(blk * F, (blk + 1) * F)
        tabs[sl, :NUNITS] = R.reshape(NUNITS, F).T
        tabs[sl, NUNITS:] = Bt.reshape(NUNITS, F).T
    return tabs


def _host_slabs(x):
    """[NCORES, NBLK*SLAB] fp16: per core, 4 block slabs [C, 14, 70]."""
    np_dt = np.float16 if _DT == "fp16" else np.float32
    out = np.zeros((NCORES, NBLK, C, HSLAB, WSLAB), np.float32)
    pad = (KH - 1) // 2
    for m in range(NCORES):
        b, half = divmod(m, 2)
        h0 = half * ROWS
        for blk in range(NBLK):
            r0 = h0 + blk * BROWS - pad
            lo, hi = max(r0, 0), min(r0 + HSLAB, H)
            out[m, blk, :, lo - r0 : hi - r0, pad : pad + W] = np.transpose(
                x[b, lo:hi, :, :], (2, 0, 1)
            )
    return out.reshape(NCORES, NBLK * SLAB).astype(np_dt)


def emulate(x, kernel, timesKernel):
    """Pure-numpy emulation of the device math (fp32; layout-faithful)."""
    tabs = _host_tables(kernel, timesKernel)
    slabs = _host_slabs(np.asarray(x, np.float32)).astype(np.float32)
    full = np.zeros((B, 4, H, W, F), np.float32)
    for m in range(NCORES):
        b, half = divmod(m, 2)
        h0 = half * ROWS
        sl = slabs[m].reshape(NBLK, C, HSLAB, WSLAB)
        acc = np.full((4, NBLK, C, F, BROWS, W), np.inf, np.float32)
        for t in range(P):
            for c in range(C):
                j = t * C + c
                r = tabs[:F, j]
                bt = tabs[:F, NUNITS + j]
                g = (
                    sl[:, c, None, :, :] * r[None, :, None, None]
                    + bt[None, :, None, None]
                )
                for i in range(4):
                    ip, jp = SH[i][t]
                    acc[i, :, c] = np.minimum(
                        acc[i, :, c], g[:, :, ip : ip + BROWS, jp : jp + W]
                    )
        o = acc.sum(axis=2)
        for blk in range(NBLK):
            full[b, :, h0 + blk * BROWS : h0 + (blk + 1) * BROWS, :, :] = (
                np.transpose(o[:, blk], (0, 2, 3, 1))
            )
    return full


def kernel(x, kernel, timesKernel):
    global last_results
    x = np.asarray(x, np.float32)
    kernel = np.asarray(kernel, np.float32)
    timesKernel = np.asarray(timesKernel, np.float32)

    tabs = _host_tables(kernel, timesKernel)
    slabs = _host_slabs(x)

    nc = _get_module()
    in_maps = [{"xs": slabs[m], "tabs": tabs} for m in range(NCORES)]
    res = run_bass_kernel_spmd(nc, in_maps, list(range(NCORES)))
    last_results = res

    full = np.zeros((B, 4, H, W, F), np.float32)
    for m in range(NCORES):
        b, half = divmod(m, 2)
        h0 = half * ROWS
        o = res.results[m]["out"].astype(np.float32).reshape(NBLK, F, 4, BROWS, W)
        for blk in range(NBLK):
            full[b, :, h0 + blk * BROWS : h0 + (blk + 1) * BROWS, :, :] = (
                np.transpose(o[blk], (1, 2, 3, 0))
            )
    return full


# revision 20
# speedup vs baseline: 5.4792x; 1.2036x over previous
"""Trainium2 Bass kernel for MaxTimesPlusErosionLiftingP4 — v2 (g-sharing).

Key idea vs v1: the four group rotations share one set of 147 affine
tap-images.  For rotation i, output pixel z:
    out_i[z] = sum_c min_q g_q[z + s_i(q)]
where g_q = (xpad - k_ero[q]) * inv_t_ero[q] (per c, f) and the shift
s_i(q) runs over the 7x7 kernel support rotated by i.  So the affine
images g_q are produced ONCE (147 ScalarE activation ops over a 14x70
per-block slab instead of 147 ops over the full 32x64 pixel range = 4x
less ScalarE work), and the per-rotation structure moves into the DVE
min-accumulate reads (strided views of g).

Device layout: 128 partitions = 4 row-blocks x 32 filters.  Each core
takes 32 of the 256 (b,h) output rows; each partition-block covers 8 of
them.  Per-partition slab = [C=3, 14, 70] fp16 (8 rows + 3 halo each
side, 64 cols + 3 pad each side).  Per tap: 3 producer ops (ScalarE
activation Identity with per-partition scale/bias; 8 whole taps run on
GpSimd tensor_scalar instead — whole taps, not scattered units, so each
tap's min waits on ONE producer engine; scattered assignment measured
~30us slower from cross-engine semaphore fan-in) then 4 DVE
min-accumulates (one per rotation, [128, 3, 8, 64] strided views of g),
emitted one tap late (software pipelining).  Channel sum in fp16 on
DVE, DMA'd out; host reassembles [B,4,H,W,F].
"""

import os
from contextlib import ExitStack

import numpy as np

import concourse.bacc as bacc
import concourse.bass as bass
import concourse.mybir as mybir
import concourse.tile as tile
from concourse.bass_utils import run_bass_kernel_spmd

B, H, W, C, F = 4, 64, 64, 3, 32
KH = KW = 7
P = KH * KW  # 49 taps
NCORES = 8
ROWS = (B * H) // NCORES  # 32 output rows per core
NBLK = 4
BROWS = ROWS // NBLK  # 8 rows per partition-block
HSLAB = BROWS + KH - 1  # 14 slab rows
WSLAB = W + KW - 1  # 70 slab cols
SLAB = C * HSLAB * WSLAB  # 2940 per partition
NUNITS = P * C  # 147
EPS = 1e-7

_DT = os.environ.get("EROSION_DT", "fp16")
_REPEAT = int(os.environ.get("EROSION_REPEAT", 1))
_GBUFS = int(os.environ.get("EROSION_GBUFS", 10))
# (removed: fusing 180-deg rotation pairs into one 5-dim tensor_tensor is
# impossible — the DVE TT ISA static pattern is TENSOR3D, max 3 free dims;
# walrus mis-folds deeper APs and the kernel crashes the exec unit)
_SUM16 = int(os.environ.get("EROSION_SUM16", 1))  # fp16 channel sum + output
_PDVE = int(os.environ.get("EROSION_PDVE", 0))  # producer units on DVE
_PGPS = int(os.environ.get("EROSION_PGPS", 24))  # producer units on GpSimd
_GPSTAP = int(os.environ.get("EROSION_GPSTAP", 8))  # whole taps on GpSimd
# (overrides _PGPS scatter: a tap's min then waits on ONE producer engine)
_SWPIPE = int(os.environ.get("EROSION_SWPIPE", 1))  # emit mins one tap late
_NOMIN = int(os.environ.get("EROSION_NOMIN", 0))  # diag: producers only
_BENCHOUT = int(os.environ.get("EROSION_BENCHOUT", 0))  # tiny output (bench only)

_cache = {}

last_results = None


def _shifts():
    """SH[i][t] = (i', j') top-left of the 8x64 view for rotation i, tap t
    (t indexes k_ero row-major: t = a*7 + b)."""
    idx = np.arange(P).reshape(KH, KW)
    sh = [[None] * P for _ in range(4)]
    for i in range(4):
        m = np.rot90(idx, i)
        for ip in range(KH):
            for jp in range(KW):
                sh[i][int(m[ip, jp])] = (ip, jp)
    return sh


SH = _shifts()


def _spread(total, count):
    return [((i + 1) * count) // total > (i * count) // total for i in range(total)]


def _build_module():
    dt = mybir.dt.float16 if _DT == "fp16" else mybir.dt.float32
    f32 = mybir.dt.float32
    sum_dt = dt if _SUM16 else f32

    nc = bacc.Bacc("TRN2", target_bir_lowering=False, debug=False)
    xs_d = nc.dram_tensor("xs", [NBLK * SLAB], dt, kind="ExternalInput")
    tabs_d = nc.dram_tensor("tabs", [128, 2 * NUNITS], f32, kind="ExternalInput")
    out_shape = [128, 64] if _BENCHOUT else [128, 4 * BROWS * W]
    out_d = nc.dram_tensor("out", out_shape, sum_dt, kind="ExternalOutput")

    prod_dve = _spread(NUNITS, _PDVE)
    prod_gps = [False] * NUNITS
    if _GPSTAP:
        for t, flag in enumerate(_spread(P, _GPSTAP)):
            if flag:
                for c in range(C):
                    prod_gps[t * C + c] = True
    else:
        rest = [j for j in range(NUNITS) if not prod_dve[j]]
        gps_in_rest = _spread(len(rest), _PGPS)
        for pos, j in enumerate(rest):
            if gps_in_rest[pos]:
                prod_gps[j] = True

    with tile.TileContext(nc) as tc, ExitStack() as ctx:
        singles = ctx.enter_context(tc.tile_pool(name="singles", bufs=1))
        gpool = ctx.enter_context(tc.tile_pool(name="g", bufs=_GBUFS))
        spool = ctx.enter_context(tc.tile_pool(name="s", bufs=2))

        slab = singles.tile([128, SLAB], dt, tag="slab", name="slab")
        tabs = singles.tile([128, 2 * NUNITS], f32, tag="tabs", name="tabs")
        # acc[:, i] = running per-channel min for rotation i
        acc = singles.tile([128, 4, C, BROWS, W], dt, tag="acc", name="acc")
        osum = singles.tile([128, 4, BROWS, W], sum_dt, tag="osum", name="osum")

        # input DMAs: per-block slab broadcast to its 32 filter partitions
        for blk in range(NBLK):
            eng = nc.sync if blk % 2 == 0 else nc.scalar
            eng.dma_start(
                out=slab[blk * F : (blk + 1) * F],
                in_=bass.AP(tensor=xs_d, offset=blk * SLAB, ap=[[0, F], [1, SLAB]]),
            )
        nc.sync.dma_start(out=tabs[:], in_=tabs_d.ap())
        rtab = tabs[:, 0:NUNITS]
        btab = tabs[:, NUNITS : 2 * NUNITS]

        slab_r = slab[:].rearrange("p (c h w) -> p c h w", c=C, h=HSLAB, w=WSLAB)

        if _NOMIN:
            nc.vector.memset(osum[:], 0.0)

        def produce(t):
            g = gpool.tile([128, SLAB], dt, tag="g", name="g")
            g_r = g[:].rearrange("p (c h w) -> p c h w", c=C, h=HSLAB, w=WSLAB)
            for c in range(C):
                j = t * C + c
                sr = rtab[:, j : j + 1]
                sb = btab[:, j : j + 1]
                if prod_dve[j]:
                    nc.vector.tensor_scalar(
                        g_r[:, c], slab_r[:, c], sr, sb,
                        mybir.AluOpType.mult, mybir.AluOpType.add,
                    )
                elif prod_gps[j]:
                    nc.gpsimd.tensor_scalar(
                        g_r[:, c], slab_r[:, c], sr, sb,
                        mybir.AluOpType.mult, mybir.AluOpType.add,
                    )
                else:
                    nc.scalar.activation(
                        out=g_r[:, c], in_=slab_r[:, c],
                        func=mybir.ActivationFunctionType.Identity,
                        bias=sb, scale=sr,
                    )
            return g_r

        for _rep in range(_REPEAT):
            first_views = [None] * 4

            def mins_of(t, g_r):
                for i in range(4):
                    ip, jp = SH[i][t]
                    src = g_r[:, :, ip : ip + BROWS, jp : jp + W]
                    if t == 0:
                        first_views[i] = src
                    elif t == 1:
                        nc.vector.tensor_tensor(
                            acc[:, i], first_views[i], src, mybir.AluOpType.min
                        )
                    else:
                        nc.vector.tensor_tensor(
                            acc[:, i], acc[:, i], src, mybir.AluOpType.min
                        )

            pending = []
            for t in range(P):
                g_r = produce(t)
                if _NOMIN:
                    continue
                pending.append((t, g_r))
                if len(pending) > _SWPIPE:
                    mins_of(*pending.pop(0))
            for args in pending:
                mins_of(*args)

            if _NOMIN:
                continue
            # channel sum: osum = acc[c0] + acc[c1] + acc[c2]
            s01 = spool.tile([128, 4, BROWS, W], sum_dt, tag="s01", name="s01")
            nc.vector.tensor_tensor(
                s01[:], acc[:, :, 0], acc[:, :, 1], mybir.AluOpType.add
            )
            nc.vector.tensor_tensor(
                osum[:], s01[:], acc[:, :, 2], mybir.AluOpType.add
            )

        osum_flat = osum[:].rearrange("p a b c -> p (a b c)")
        if _BENCHOUT:
            nc.sync.dma_start(out=out_d.ap(), in_=osum_flat[:, :64])
        else:
            nc.sync.dma_start(out=out_d.ap(), in_=osum_flat)

    nc.compile()
    return nc


def _get_module():
    key = (_DT, _REPEAT, _GBUFS, _SUM16, _PDVE, _PGPS, _NOMIN, _BENCHOUT)
    if key not in _cache:
        _cache[key] = _build_module()
    return _cache[key]


def _host_tables(kernel, timesKernel):
    """tabs[p, j] = r; tabs[p, 147+j] = -k*r for unit j = t*C + c,
    t in k_ero row-major coords; p = blk*32 + f (f-dependent only)."""
    k_ero = np.rot90(kernel, 2, axes=(0, 1)).reshape(P, C, F)
    t_ero = np.rot90(timesKernel, 2, axes=(0, 1)).reshape(P, C, F)
    R = (1.0 / (t_ero + np.float32(EPS))).astype(np.float32)  # [P,C,F]
    Bt = (-k_ero * R).astype(np.float32)
    tabs = np.zeros((128, 2 * NUNITS), np.float32)
    for blk in range(NBLK):
        sl = slice(blk * F, (blk + 1) * F)
        tabs[sl, :NUNITS] = R.reshape(NUNITS, F).T
        tabs[sl, NUNITS:] = Bt.reshape(NUNITS, F).T
    return tabs


def _host_slabs(x):
    """[NCORES, NBLK*SLAB] fp16: per core, 4 block slabs [C, 14, 70]."""
    np_dt = np.float16 if _DT == "fp16" else np.float32
    out = np.zeros((NCORES, NBLK, C, HSLAB, WSLAB), np.float32)
    pad = (KH - 1) // 2
    for m in range(NCORES):
        b, half = divmod(m, 2)
        h0 = half * ROWS
        for blk in range(NBLK):
            r0 = h0 + blk * BROWS - pad
            lo, hi = max(r0, 0), min(r0 + HSLAB, H)
            out[m, blk, :, lo - r0 : hi - r0, pad : pad + W] = np.transpose(
                x[b, lo:hi, :, :], (2, 0, 1)
            )
    return out.reshape(NCORES, NBLK * SLAB).astype(np_dt)


def emulate(x, kernel, timesKernel):
    """Pure-numpy emulation of the device math (fp32; layout-faithful)."""
    tabs = _host_tables(kernel, timesKernel)
    slabs = _host_slabs(np.asarray(x, np.float32)).astype(np.float32)
    full = np.zeros((B, 4, H, W, F), np.float32)
    for m in range(NCORES):
        b, half = divmod(m, 2)
        h0 = half * ROWS
        sl = slabs[m].reshape(NBLK, C, HSLAB, WSLAB)
        acc = np.full((4, NBLK, C, F, BROWS, W), np.inf, np.float32)
        for t in range(P):
            for c in range(C):
                j = t * C + c
                r = tabs[:F, j]
                bt = tabs[:F, NUNITS + j]
                g = (
                    sl[:, c, None, :, :] * r[None, :, None, None]
                    + bt[None, :, None, None]
                )
                for i in range(4):
                    ip, jp = SH[i][t]
                    acc[i, :, c] = np.minimum(
                        acc[i, :, c], g[:, :, ip : ip + BROWS, jp : jp + W]
                    )
        o = acc.sum(axis=2)
        for blk in range(NBLK):
            full[b, :, h0 + blk * BROWS : h0 + (blk + 1) * BROWS, :, :] = (
                np.transpose(o[:, blk], (0, 2, 3, 1))
            )
    return full


def kernel(x, kernel, timesKernel):
    global last_results
    x = np.asarray(x, np.float32)
    kernel = np.asarray(kernel, np.float32)
    timesKernel = np.asarray(timesKernel, np.float32)

    tabs = _host_tables(kernel, timesKernel)
    slabs = _host_slabs(x)

    nc = _get_module()
    in_maps = [{"xs": slabs[m], "tabs": tabs} for m in range(NCORES)]
    res = run_bass_kernel_spmd(nc, in_maps, list(range(NCORES)))
    last_results = res

    full = np.zeros((B, 4, H, W, F), np.float32)
    for m in range(NCORES):
        b, half = divmod(m, 2)
        h0 = half * ROWS
        o = res.results[m]["out"].astype(np.float32).reshape(NBLK, F, 4, BROWS, W)
        for blk in range(NBLK):
            full[b, :, h0 + blk * BROWS : h0 + (blk + 1) * BROWS, :, :] = (
                np.transpose(o[blk], (1, 2, 3, 0))
            )
    return full
